# revision 11
# baseline (speedup 1.0000x reference)
"""Trainium2 Bass kernel for nn_ObjectRetriever (ragged_sequence).

Decomposition (validated against reference in golden.py):
  * Cross-attention queries are per-frame (objects of a frame share the same
    image feature => same query => same cross-attn output). Stage A collapses
    to a 64-query x N-key attention. K/V projections sharded over objects
    (N/8 per core); softmax without max-subtraction (scores are small), so
    per-core partial numerators/denominators combine exactly via AllGather.
  * Ragged window packing resolved on host at trace time (kernel is
    JIT-specialized per input). Per-core behavior driven purely by input
    data (packed objects, one-hot "sel" matrices injecting cross/pos rows,
    mask-bias columns) so one SPMD NEFF runs on all 8 cores.
  * Stage C: 64 windows (62 real + 2 dummy), 8/core, 2 encoder layers in
    bf16 matmuls with fp32 PSUM/softmax/LN math. Activations feature-major
    [E, T]; LN partition-reductions via ones-matmuls; biases folded into
    per-partition activation bias or K=1 ones-row matmuls; key-padding mask
    folded into the Exp activation bias. Windows padded to 256 keys via
    overlapping reads (masked), so all attention matmuls are dense.
"""

import numpy as np
from contextlib import ExitStack

NHEAD = 4
E = 1024
HD = E // NHEAD          # 256
IMG = 2048
DFF = 2048
N_LAYERS = 2
N_CORES = 8
L_WIN = 192
LK = 256                 # padded key length per window
EC = E // 128            # 8
FC = DFF // 128          # 16
IC = IMG // 128          # 16

_CACHE = {}


def _host_metadata(obj_idx, bF, N):
    """Mirror reference.py lines 86-101 exactly."""
    c = np.bincount(obj_idx, minlength=bF)
    cum = np.concatenate([[0], np.cumsum(c)]).astype(np.int64)
    nw = bF - 2
    slots = np.arange(L_WIN)[:, None]
    wins = np.arange(nw)[None, :]
    start = cum[wins]
    lenw = cum[wins + 3] - start
    valid = slots < lenw
    g = np.clip(start + slots, 0, N - 1)
    t1 = c[wins]
    t2 = t1 + c[wins + 1]
    pid = np.where(slots < t1, 0, np.where(slots < t2, 1, 2))
    return c, cum, nw, valid, g, pid


def _build(host):
    import concourse.bacc as bacc
    import concourse.tile as tile
    import concourse.mybir as mybir

    dt = mybir.dt
    AF = mybir.ActivationFunctionType
    ALU = mybir.AluOpType
    f32, f32r, bf16 = dt.float32, dt.float32r, dt.bfloat16

    NW = host["NW"]                  # windows per core (8)
    T = NW * L_WIN                   # 1536
    TPAD = T + (LK - L_WIN)          # 1600
    NS = host["NS"]                  # 512
    bF = host["bF"]                  # 64
    NT = T // 512                    # 3
    NWH = NW // 2                    # 4 windows per half
    TH = NWH * L_WIN                 # 768
    THP = TH + 64                    # 832
    PROWS = 1152
    PCN = host["pcol_ncols"]

    nc = bacc.Bacc("TRN2", target_bir_lowering=False, debug=False,
                   enable_asserts=False, num_devices=N_CORES)

    def din(name, shape, dtype=f32):
        return nc.dram_tensor(name, list(shape), dtype, kind="ExternalInput").ap()

    objT = din("objT", (E, NS))
    imgT = din("imgT", (IMG, bF))
    pobjT = din("pobjT", (E, T))
    selT = din("selT", (68, T), bf16)
    maskcol = din("maskcol", (128, NW * 2))
    pcol = din("pcol", (128, PCN))
    posb = din("posb", (4, E), bf16)
    wi2oT = din("wi2oT", (IMG, E))
    wqT_A = din("wqT_A", (E, E))
    wkT_A = din("wkT_A", (E, E))
    wvT_A = din("wvT_A", (E, E))
    bvrow_A = din("bvrow_A", (1, E))
    coutT = din("coutT", (E, E))
    qkw = [din(f"qkw{l}", (EC, EC, 128, 256), bf16) for l in range(N_LAYERS)]
    vw4 = [din(f"vw{l}", (2, EC, 128, 512), bf16) for l in range(N_LAYERS)]
    bvrow = [din(f"bvrow{l}", (1, E), bf16) for l in range(N_LAYERS)]
    ow4 = [din(f"ow{l}", (EC, EC, 128, 128), bf16) for l in range(N_LAYERS)]
    l1w4 = [din(f"l1w{l}", (FC, EC, 128, 128), bf16) for l in range(N_LAYERS)]
    l2w4 = [din(f"l2w{l}", (EC, FC, 128, 128), bf16) for l in range(N_LAYERS)]

    outD = nc.dram_tensor("outD", [E, T], f32, kind="ExternalOutput").ap()

    PC = host["pcol_idx"]

    with tile.TileContext(nc) as tc, ExitStack() as ctx:
        const = ctx.enter_context(tc.tile_pool(name="const", bufs=1))
        psum = ctx.enter_context(tc.tile_pool(name="ps", bufs=1, space="PSUM"))
        dram = ctx.enter_context(tc.tile_pool(name="dram", bufs=1, space="DRAM"))

        def ps_big():
            return psum.tile([128, 512], f32, tag="big", bufs=4, name="psbig")

        def ps_small():
            return psum.tile([128, 256], f32, tag="small", bufs=4,
                             name="pssmall")

        pcol_sb = const.tile([128, PCN], f32, name="pcol_sb")
        nc.sync.dma_start(pcol_sb[:], pcol[:])
        mask_sb = const.tile([128, NW * 2], f32, name="mask_sb")
        nc.sync.dma_start(mask_sb[:], maskcol[:])
        ones_f = const.tile([128, 128], f32, name="ones_f")
        nc.gpsimd.memset(ones_f[:], 1.0)
        ones_b = const.tile([128, 128], bf16, name="ones_b")
        nc.gpsimd.memset(ones_b[:], 1.0)
        ones_r1 = const.tile([1, 128], f32, name="ones_r1")
        nc.gpsimd.memset(ones_r1[:], 1.0)
        ones_r1b = const.tile([1, 128], bf16, name="ones_r1b")
        nc.gpsimd.memset(ones_r1b[:], 1.0)
        concat_sb = const.tile([68, E], bf16, name="concat_sb")
        selT_sb = const.tile([68, T], bf16, name="selT_sb")

        def pc(name, j=0):
            i = PC[name] + j
            return pcol_sb[:, i:i + 1]

        def mm(out, lhsT, rhs, start, stop):
            nc.tensor.matmul(out, lhsT, rhs, start=start, stop=stop)

        # ================= Stage A: per-frame cross attention =============
        with tc.tile_pool(name="stageA", bufs=1) as pA, \
             tc.tile_pool(name="wstream", bufs=3) as wp:
            imgT_sb = pA.tile([128, IC * bF], f32, name="imgT_sb")
            for k in range(IC):
                nc.sync.dma_start(imgT_sb[:, k * bF:(k + 1) * bF],
                                  imgT[k * 128:(k + 1) * 128, :])
            objT_sb = pA.tile([128, EC * NS], f32, name="objT_sb")
            for k in range(EC):
                nc.sync.dma_start(objT_sb[:, k * NS:(k + 1) * NS],
                                  objT[k * 128:(k + 1) * 128, :])

            imgp_sb = pA.tile([128, EC * bF], f32, name="imgp_sb")
            for m in range(EC):
                ps = ps_small()
                for k in range(IC):
                    w = wp.tile([128, 128], f32, tag="w", name="wA")
                    nc.sync.dma_start(w[:], wi2oT[k * 128:(k + 1) * 128,
                                                  m * 128:(m + 1) * 128])
                    mm(ps[:, 0:bF], w[:], imgT_sb[:, k * bF:(k + 1) * bF],
                       start=(k == 0), stop=(k == IC - 1))
                nc.scalar.activation(imgp_sb[:, m * bF:(m + 1) * bF],
                                     ps[:, 0:bF], AF.Identity,
                                     bias=pc("b_i2o", m))
            qA_sb = pA.tile([128, EC * bF], f32, name="qA_sb")
            for m in range(EC):
                ps = ps_small()
                for k in range(EC):
                    w = wp.tile([128, 128], f32, tag="w", name="wA")
                    nc.sync.dma_start(w[:], wqT_A[k * 128:(k + 1) * 128,
                                                  m * 128:(m + 1) * 128])
                    mm(ps[:, 0:bF], w[:], imgp_sb[:, k * bF:(k + 1) * bF],
                       start=(k == 0), stop=(k == EC - 1))
                nc.scalar.activation(qA_sb[:, m * bF:(m + 1) * bF],
                                     ps[:, 0:bF], AF.Identity,
                                     bias=pc("bq_A", m))

            kA_sb = pA.tile([128, EC * NS], f32, name="kA_sb")
            for m in range(EC):
                ps = ps_big()
                for k in range(EC):
                    w = wp.tile([128, 128], f32, tag="w", name="wA")
                    nc.sync.dma_start(w[:], wkT_A[k * 128:(k + 1) * 128,
                                                  m * 128:(m + 1) * 128])
                    mm(ps[:, 0:NS], w[:], objT_sb[:, k * NS:(k + 1) * NS],
                       start=(k == 0), stop=(k == EC - 1))
                nc.scalar.activation(kA_sb[:, m * NS:(m + 1) * NS],
                                     ps[:, 0:NS], AF.Identity,
                                     bias=pc("bk_A", m))

            vA_sb = pA.tile([128, 4 * E], f32, name="vA_sb")
            bvA_sb = pA.tile([1, E], f32, name="bvA_sb")
            nc.sync.dma_start(bvA_sb[:], bvrow_A[:])
            for mt in range(NS // 128):
                for n in range(E // 512):
                    ps = ps_big()
                    for k in range(EC):
                        w = wp.tile([128, 512], f32, tag="wv", name="wAv")
                        nc.sync.dma_start(w[:], wvT_A[k * 128:(k + 1) * 128,
                                                      n * 512:(n + 1) * 512])
                        mm(ps[:], objT_sb[:, k * NS + mt * 128:
                                          k * NS + (mt + 1) * 128], w[:],
                           start=(k == 0), stop=False)
                    mm(ps[:], ones_r1[:], bvA_sb[:, n * 512:(n + 1) * 512],
                       start=False, stop=True)
                    nc.scalar.activation(vA_sb[:, mt * E + n * 512:
                                               mt * E + (n + 1) * 512],
                                         ps[:], AF.Copy)

            pA_sb = pA.tile([128, 4 * NHEAD * bF], f32, name="pA_sb")
            for ot in range(NS // 128):
                for h in range(NHEAD):
                    ps = ps_small()
                    for kc in range(HD // 128):
                        c = h * 2 + kc
                        mm(ps[:, 0:bF],
                           kA_sb[:, c * NS + ot * 128:c * NS + (ot + 1) * 128],
                           qA_sb[:, c * bF:(c + 1) * bF],
                           start=(kc == 0), stop=(kc == 1))
                    nc.scalar.activation(
                        pA_sb[:, (ot * NHEAD + h) * bF:(ot * NHEAD + h + 1) * bF],
                        ps[:, 0:bF], AF.Exp)

            partO_sb = pA.tile([128, EC * bF], f32, name="partO_sb")
            for h in range(NHEAD):
                for m2 in range(HD // 128):
                    ps = ps_small()
                    for ot in range(NS // 128):
                        mm(ps[:, 0:bF],
                           vA_sb[:, ot * E + h * HD + m2 * 128:
                                 ot * E + h * HD + (m2 + 1) * 128],
                           pA_sb[:, (ot * NHEAD + h) * bF:
                                 (ot * NHEAD + h + 1) * bF],
                           start=(ot == 0), stop=(ot == 3))
                    nc.scalar.activation(
                        partO_sb[:, (h * 2 + m2) * bF:(h * 2 + m2 + 1) * bF],
                        ps[:, 0:bF], AF.Copy)
            partD_sb = pA.tile([1, NHEAD * bF], f32, name="partD_sb")
            for h in range(NHEAD):
                ps = ps_small()
                for ot in range(NS // 128):
                    mm(ps[0:1, 0:bF], ones_f[:, 0:1],
                       pA_sb[:, (ot * NHEAD + h) * bF:(ot * NHEAD + h + 1) * bF],
                       start=(ot == 0), stop=(ot == 3))
                nc.scalar.activation(partD_sb[:, h * bF:(h + 1) * bF],
                                     ps[0:1, 0:bF], AF.Copy)

            part_d = dram.tile([PROWS, bF], f32, name="part_d")
            gath_d = dram.tile([N_CORES * PROWS, bF], f32,
                               addr_space="Shared", name="gath_d")
            nc.sync.dma_start(
                part_d[0:E, :].rearrange("(g p) f -> p g f", p=128),
                partO_sb[:].rearrange("p (g f) -> p g f", f=bF))
            nc.sync.dma_start(
                part_d[E:E + NHEAD, :].rearrange("(o h) f -> o (h f)", o=1),
                partD_sb[:])
            nc.gpsimd.collective_compute(
                "AllGather", ALU.bypass,
                replica_groups=[list(range(N_CORES))],
                ins=[part_d.opt()], outs=[gath_d.opt()])

            accO_sb = pA.tile([128, EC * bF], f32, name="accO_sb")
            accD_sb = pA.tile([1, NHEAD * bF], f32, name="accD_sb")
            tmpO = pA.tile([128, EC * bF], f32, name="tmpO", bufs=2)
            tmpD = pA.tile([1, NHEAD * bF], f32, name="tmpD", bufs=2)
            for cc in range(N_CORES):
                dstO = accO_sb if cc == 0 else tmpO
                dstD = accD_sb if cc == 0 else tmpD
                if cc > 1:
                    tmpO = pA.tile([128, EC * bF], f32, name="tmpO",
                                   tag="tmpO", bufs=2)
                    tmpD = pA.tile([1, NHEAD * bF], f32, name="tmpD",
                                   tag="tmpD", bufs=2)
                    dstO, dstD = tmpO, tmpD
                base = cc * PROWS
                nc.sync.dma_start(
                    dstO[:].rearrange("p (g f) -> p g f", f=bF),
                    gath_d[base:base + E, :].rearrange("(g p) f -> p g f",
                                                       p=128))
                nc.sync.dma_start(
                    dstD[:],
                    gath_d[base + E:base + E + NHEAD, :]
                    .rearrange("(o h) f -> o (h f)", o=1))
                if cc > 0:
                    nc.vector.tensor_add(accO_sb[:], accO_sb[:], dstO[:])
                    nc.vector.tensor_add(accD_sb[:], accD_sb[:], dstD[:])

            recD_sb = pA.tile([1, NHEAD * bF], f32, name="recD_sb")
            nc.vector.reciprocal(recD_sb[:], accD_sb[:])
            recB_sb = pA.tile([128, NHEAD * bF], f32, name="recB_sb")
            for h in range(NHEAD):
                ps = ps_small()
                mm(ps[:, 0:bF], ones_r1[:], recD_sb[:, h * bF:(h + 1) * bF],
                   start=True, stop=True)
                nc.scalar.activation(recB_sb[:, h * bF:(h + 1) * bF],
                                     ps[:, 0:bF], AF.Copy)
            crossT_sb = pA.tile([128, EC * bF], f32, name="crossT_sb")
            for h in range(NHEAD):
                for m2 in range(HD // 128):
                    gi = h * 2 + m2
                    nc.vector.tensor_mul(
                        crossT_sb[:, gi * bF:(gi + 1) * bF],
                        accO_sb[:, gi * bF:(gi + 1) * bF],
                        recB_sb[:, h * bF:(h + 1) * bF])

            nc.sync.dma_start(concat_sb[64:68, :], posb[:])
            for n in range(E // 512):
                ps = ps_big()
                for k in range(EC):
                    w = wp.tile([128, 512], f32, tag="wv", name="wAv")
                    nc.sync.dma_start(w[:], coutT[k * 128:(k + 1) * 128,
                                                  n * 512:(n + 1) * 512])
                    mm(ps[0:64, :], crossT_sb[:, k * bF:(k + 1) * bF], w[:],
                       start=(k == 0), stop=(k == EC - 1))
                nc.scalar.activation(concat_sb[0:64, n * 512:(n + 1) * 512],
                                     ps[0:64, :], AF.Copy)

        # ================= Phase B: build x =================
        nc.sync.dma_start(selT_sb[:], selT[:])
        xpool = ctx.enter_context(tc.tile_pool(name="xpool", bufs=1))
        x_sb = []
        with tc.tile_pool(name="pobj", bufs=2) as pop:
            for k in range(EC):
                xt = xpool.tile([128, TPAD], bf16, tag=f"x{k}", name=f"x{k}")
                po = pop.tile([128, T], f32, tag="po", name="po")
                nc.sync.dma_start(po[:], pobjT[k * 128:(k + 1) * 128, :])
                for w in range(NW):
                    ps = ps_small()
                    mm(ps[:, 0:L_WIN], concat_sb[:, k * 128:(k + 1) * 128],
                       selT_sb[:, w * L_WIN:(w + 1) * L_WIN],
                       start=True, stop=True)
                    nc.vector.tensor_add(
                        xt[:, w * L_WIN:(w + 1) * L_WIN], ps[:, 0:L_WIN],
                        po[:, w * L_WIN:(w + 1) * L_WIN])
                nc.gpsimd.memset(xt[:, T:TPAD], 0.0)
                x_sb.append(xt)

        # ================= Stage C =================
        at_pool = ctx.enter_context(tc.tile_pool(name="at", bufs=1))
        ln_pool = ctx.enter_context(tc.tile_pool(name="ln", bufs=1))

        def layernorm(pre, dst, g_name, b_name):
            for tt in range(NT):
                s0, s1 = tt * 512, (tt + 1) * 512
                psm = ps_big()
                for k in range(EC):
                    mm(psm[:], ones_b[:], pre[k][:, s0:s1],
                       start=(k == 0), stop=(k == EC - 1))
                pss = ps_big()
                for k in range(EC):
                    sq = ln_pool.tile([128, 512], bf16, tag="lnsq", bufs=2,
                                      name="lnsq")
                    nc.scalar.activation(sq[:], pre[k][:, s0:s1], AF.Square)
                    mm(pss[:], ones_b[:], sq[:],
                       start=(k == 0), stop=(k == EC - 1))
                var = ln_pool.tile([128, 512], f32, tag="lnvar", name="lnvar")
                nc.vector.tensor_scalar_mul(var[:], psm[:], 1.0 / E)
                nc.vector.tensor_mul(var[:], var[:], var[:])
                nc.vector.scalar_tensor_tensor(
                    var[:], pss[:], 1.0 / E, var[:],
                    op0=ALU.mult, op1=ALU.subtract)
                rstd = ln_pool.tile([128, 512], f32, tag="lnrstd", bufs=2,
                                    name="lnrstd")
                nc.scalar.activation(rstd[:], var[:], AF.Sqrt, bias=pc("eps"))
                nc.vector.reciprocal(rstd[:], rstd[:])
                for k in range(EC):
                    d = ln_pool.tile([128, 512], f32, tag="lnd", bufs=2,
                                     name="lnd")
                    nc.vector.scalar_tensor_tensor(
                        d[:], psm[:], -1.0 / E, pre[k][:, s0:s1],
                        op0=ALU.mult, op1=ALU.add)
                    nc.vector.tensor_mul(d[:], d[:], rstd[:])
                    nc.vector.tensor_scalar(
                        dst[k][:, s0:s1], d[:],
                        pc(g_name, k), pc(b_name, k),
                        op0=ALU.mult, op1=ALU.add)

        cur = x_sb
        for l in range(N_LAYERS):
            with tc.tile_pool(name=f"rs{l}", bufs=1) as rs_pool:
                pre = [rs_pool.tile([128, T], bf16, tag=f"pre{m}",
                                    name=f"pre{m}_{l}") for m in range(EC)]
                src = [rs_pool.tile([128, T], bf16, tag=f"s{m}",
                                    name=f"s{m}_{l}") for m in range(EC)]
                attnT = [at_pool.tile([128, T], bf16, tag=f"a{m}",
                                      name=f"a{m}_{l}") for m in range(EC)]
                with tc.tile_pool(name=f"qkv{l}", bufs=1) as qk_pool, \
                     tc.tile_pool(name=f"pw{l}", bufs=4) as p_pool, \
                     tc.tile_pool(name=f"wat{l}", bufs=1) as wat:
                    bv_sb = wat.tile([1, E], bf16, name="bvr")
                    nc.sync.dma_start(bv_sb[:], bvrow[l][:])
                    for hf in range(2):
                        hb = hf * TH
                        qT = [qk_pool.tile([128, THP], bf16, tag=f"qh{m}",
                                           name=f"qh{m}_{l}{hf}")
                              for m in range(EC)]
                        kT = [qk_pool.tile([128, THP], bf16, tag=f"kh{m}",
                                           name=f"kh{m}_{l}{hf}")
                              for m in range(EC)]
                        for m in range(EC):
                            ws = []
                            for k in range(EC):
                                w = wat.tile([128, 256], bf16, tag="wqk",
                                             bufs=8, name="wqk")
                                nc.sync.dma_start(w[:], qkw[l][k, m])
                                ws.append(w)
                            for tp in range(2):
                                t0 = tp * 384
                                psq = ps_big()
                                psk = ps_big()
                                for k in range(EC):
                                    mm(psq[:, 0:384], ws[k][:, 0:128],
                                       cur[k][:, hb + t0:hb + t0 + 384],
                                       start=(k == 0), stop=(k == EC - 1))
                                    mm(psk[:, 0:384], ws[k][:, 128:256],
                                       cur[k][:, hb + t0:hb + t0 + 384],
                                       start=(k == 0), stop=(k == EC - 1))
                                nc.scalar.activation(
                                    qT[m][:, t0:t0 + 384], psq[:, 0:384],
                                    AF.Identity, bias=pc(f"bq{l}", m))
                                nc.scalar.activation(
                                    kT[m][:, t0:t0 + 384], psk[:, 0:384],
                                    AF.Identity, bias=pc(f"bk{l}", m))
                            nc.gpsimd.memset(qT[m][:, TH:THP], 0.0)
                            nc.gpsimd.memset(kT[m][:, TH:THP], 0.0)

                        vh = qk_pool.tile([128, NWH * 2 * E], bf16, tag="vh",
                                          name=f"vh_{l}{hf}")
                        for n in range(2):
                            wvs = []
                            for k in range(EC):
                                w = wat.tile([128, 512], bf16, tag="wvv",
                                             bufs=8, name="wvv")
                                nc.sync.dma_start(w[:], vw4[l][n, k])
                                wvs.append(w)
                            for mt in range(NWH * 2):
                                wl, ot = mt // 2, mt % 2
                                ps = ps_big()
                                for k in range(EC):
                                    mm(ps[:],
                                       cur[k][:, hb + wl * L_WIN + ot * 128:
                                              hb + wl * L_WIN + (ot + 1) * 128],
                                       wvs[k][:], start=(k == 0), stop=False)
                                mm(ps[:], ones_r1b[:],
                                   bv_sb[:, n * 512:(n + 1) * 512],
                                   start=False, stop=True)
                                nc.scalar.activation(
                                    vh[:, mt * E + n * 512:mt * E + (n + 1) * 512],
                                    ps[:], AF.Copy)

                        for wl in range(NWH):
                            w_ = hf * NWH + wl
                            base = wl * L_WIN
                            for h in range(NHEAD):
                                pw = p_pool.tile([128, 2 * LK], bf16,
                                                 tag="pw", name="pw")
                                psd = ps_small()
                                for ot in range(2):
                                    pss = ps_small()
                                    for kc in range(HD // 128):
                                        c = h * 2 + kc
                                        mm(pss[:],
                                           kT[c][:, base + ot * 128:
                                                 base + (ot + 1) * 128],
                                           qT[c][:, base:base + LK],
                                           start=(kc == 0), stop=(kc == 1))
                                    nc.scalar.activation(
                                        pw[:, ot * LK:(ot + 1) * LK], pss[:],
                                        AF.Exp,
                                        bias=mask_sb[:, w_ * 2 + ot:
                                                     w_ * 2 + ot + 1])
                                    mm(psd[:], ones_b[:],
                                       pw[:, ot * LK:(ot + 1) * LK],
                                       start=(ot == 0), stop=(ot == 1))
                                drec = ln_pool.tile([128, LK], f32,
                                                    tag="drec", bufs=2,
                                                    name="drec")
                                nc.vector.reciprocal(drec[:], psd[:])
                                for m2 in range(HD // 128):
                                    pso = ps_small()
                                    for ot in range(2):
                                        mm(pso[:],
                                           vh[:, (wl * 2 + ot) * E + h * HD +
                                              m2 * 128:
                                              (wl * 2 + ot) * E + h * HD +
                                              (m2 + 1) * 128],
                                           pw[:, ot * LK:(ot + 1) * LK],
                                           start=(ot == 0), stop=(ot == 1))
                                    nc.vector.tensor_mul(
                                        attnT[h * 2 + m2][:, w_ * L_WIN:
                                                          (w_ + 1) * L_WIN],
                                        pso[:, 0:L_WIN], drec[:, 0:L_WIN])

                    # ---- out-proj + residual -> pre ----
                    for m in range(EC):
                        ws = []
                        for k in range(EC):
                            w = wat.tile([128, 128], bf16, tag="wo", bufs=8,
                                         name="wo")
                            nc.sync.dma_start(w[:], ow4[l][k, m])
                            ws.append(w)
                        for tt in range(NT):
                            s0, s1 = tt * 512, (tt + 1) * 512
                            ps = ps_big()
                            for k in range(EC):
                                mm(ps[:], ws[k][:], attnT[k][:, s0:s1],
                                   start=(k == 0), stop=(k == EC - 1))
                            nc.vector.scalar_tensor_tensor(
                                pre[m][:, s0:s1], ps[:], pc(f"bo{l}", m),
                                cur[m][:, s0:s1], op0=ALU.add, op1=ALU.add)

                layernorm(pre, src, f"n1g{l}", f"n1b{l}")

                # ---- FFN ----
                with tc.tile_pool(name=f"ffn{l}", bufs=1) as fp, \
                     tc.tile_pool(name=f"wf{l}", bufs=1) as wf:
                    h1 = [fp.tile([128, T], bf16, tag=f"h1_{m}",
                                  name=f"h1_{m}_{l}") for m in range(FC)]
                    for m in range(FC):
                        ws = []
                        for k in range(EC):
                            w = wf.tile([128, 128], bf16, tag="wf1", bufs=16,
                                        name="wf1")
                            nc.sync.dma_start(w[:], l1w4[l][m, k])
                            ws.append(w)
                        for tt in range(NT):
                            s0, s1 = tt * 512, (tt + 1) * 512
                            ps = ps_big()
                            for k in range(EC):
                                mm(ps[:], ws[k][:], src[k][:, s0:s1],
                                   start=(k == 0), stop=(k == EC - 1))
                            nc.scalar.activation(h1[m][:, s0:s1], ps[:],
                                                 AF.Relu, bias=pc(f"b1{l}", m))
                    for m in range(EC):
                        ws = []
                        for k in range(FC):
                            w = wf.tile([128, 128], bf16, tag="wf2", bufs=32,
                                        name="wf2")
                            nc.sync.dma_start(w[:], l2w4[l][m, k])
                            ws.append(w)
                        for tt in range(NT):
                            s0, s1 = tt * 512, (tt + 1) * 512
                            ps = ps_big()
                            for k in range(FC):
                                mm(ps[:], ws[k][:], h1[k][:, s0:s1],
                                   start=(k == 0), stop=(k == FC - 1))
                            nc.vector.scalar_tensor_tensor(
                                pre[m][:, s0:s1], ps[:], pc(f"b2{l}", m),
                                src[m][:, s0:s1], op0=ALU.add, op1=ALU.add)

                if l < N_LAYERS - 1:
                    nxt = [xpool.tile([128, TPAD], bf16, tag=f"x{k}",
                                      name=f"x{k}_n{l}") for k in range(EC)]
                    layernorm(pre, nxt, f"n2g{l}", f"n2b{l}")
                    for k in range(EC):
                        nc.gpsimd.memset(nxt[k][:, T:TPAD], 0.0)
                    cur = nxt
                else:
                    with tc.tile_pool(name="outf", bufs=1) as out_pool:
                        fin = [out_pool.tile([128, T], f32, tag=f"fo{k}",
                                             name=f"fo{k}") for k in range(EC)]
                        layernorm(pre, fin, f"n2g{l}", f"n2b{l}")
                        for k in range(EC):
                            nc.sync.dma_start(outD[k * 128:(k + 1) * 128, :],
                                              fin[k][:])

    nc.compile()
    return nc


def _prepare(inputs):
    import ml_dtypes
    bfl = lambda a: np.ascontiguousarray(a).astype(ml_dtypes.bfloat16)
    tr = lambda a: np.ascontiguousarray(np.asarray(a).T.astype(np.float32))

    obj = np.asarray(inputs['obj_features'], np.float32)
    img = np.asarray(inputs['img_features'], np.float32)
    obj_idx = np.asarray(inputs['obj_idx']).astype(np.int64)
    N = obj.shape[0]
    bF = img.shape[0]

    c, cum, nw, valid, g, pid = _host_metadata(obj_idx, bF, N)
    NW = -(-nw // N_CORES)              # 8
    NWT = NW * N_CORES                  # 64
    NS = N // N_CORES
    T = NW * L_WIN

    packed = np.zeros((NWT, L_WIN, E), np.float32)
    sel = np.zeros((NWT, L_WIN, 68), np.float32)
    maskb = np.zeros((NWT, 2 * 128), np.float32)
    sl = np.arange(L_WIN)
    for w in range(nw):
        vm = valid[:, w]
        packed[w][vm] = 0.5 * obj[g[vm, w], :]
        fr = obj_idx[g[:, w]]
        sel[w, sl[vm], fr[vm]] += 1.0
        sel[w, sl[vm], 64 + pid[vm, w]] += 1.0
        sel[w, vm, 67] = 1.0
        mrow = np.full(2 * 128, -1e9, np.float32)
        mrow[:L_WIN][vm] = 0.0
        maskb[w] = mrow
    for w in range(nw, NWT):
        mrow = np.full(2 * 128, -1e9, np.float32)
        mrow[:L_WIN] = 0.0
        maskb[w] = mrow

    scale = np.float32(1.0 / np.sqrt(HD))
    cin_w = np.asarray(inputs['cross_in_w'], np.float32)
    cin_b = np.asarray(inputs['cross_in_b'], np.float32)
    wq, wk, wv = np.split(cin_w, 3, axis=0)
    bq, bk, bv = np.split(cin_b, 3)

    def col(vec):
        return np.ascontiguousarray(np.asarray(vec, np.float32)
                                    .reshape(-1, 128).T)

    pcol_parts = []
    pcol_idx = {}

    def add_pc(name, vec):
        pcol_idx[name] = sum(p.shape[1] for p in pcol_parts)
        pcol_parts.append(col(vec))

    add_pc("b_i2o", inputs['b_i2o'])
    add_pc("bq_A", bq * scale)
    add_pc("bk_A", bk)
    add_pc("eps", np.full(128, 1e-5, np.float32))
    for l in range(N_LAYERS):
        qkv_b = np.asarray(inputs['sa_in_b'][l], np.float32)
        add_pc(f"bq{l}", qkv_b[:E] * scale)
        add_pc(f"bk{l}", qkv_b[E:2 * E])
        add_pc(f"bo{l}", inputs['sa_out_b'][l])
        add_pc(f"b1{l}", inputs['sa_l1_b'][l])
        add_pc(f"b2{l}", inputs['sa_l2_b'][l])
        add_pc(f"n1g{l}", inputs['sa_n1_g'][l])
        add_pc(f"n1b{l}", inputs['sa_n1_b'][l])
        add_pc(f"n2g{l}", inputs['sa_n2_g'][l])
        add_pc(f"n2b{l}", inputs['sa_n2_b'][l])
    pcol = np.ascontiguousarray(np.concatenate(pcol_parts, axis=1))

    def tile4(wt, kt, mt, mw):
        """wt: [K, M] (already transposed) -> [K/128, M/mw, 128, mw]."""
        K, M = wt.shape
        return np.ascontiguousarray(
            wt.reshape(K // 128, 128, M // mw, mw).transpose(0, 2, 1, 3))

    shared = {
        "imgT": tr(img),
        "pcol": pcol,
        "posb": bfl(np.concatenate(
            [np.asarray(inputs['pos_emb'], np.float32),
             (0.5 * np.asarray(inputs['cross_out_b'], np.float32))[None, :]],
            axis=0)),
        "wi2oT": tr(inputs['w_i2o']),
        "wqT_A": tr(wq * scale),
        "wkT_A": tr(wk),
        "wvT_A": tr(wv),
        "bvrow_A": np.ascontiguousarray(bv[None, :].astype(np.float32)),
        "coutT": tr(0.5 * np.asarray(inputs['cross_out_w'], np.float32)),
    }
    for l in range(N_LAYERS):
        qkvw = np.asarray(inputs['sa_in_w'][l], np.float32).copy()
        qkvw[:E] *= scale
        qkvT = qkvw.T                        # [E, 3E]
        # qk tiles: [k, m, 128, 256] with cols 0:128 = q-part m, 128:256 = k
        qk_t = np.empty((EC, EC, 128, 256), np.float32)
        for k in range(EC):
            for m in range(EC):
                qk_t[k, m, :, 0:128] = qkvT[k * 128:(k + 1) * 128,
                                            m * 128:(m + 1) * 128]
                qk_t[k, m, :, 128:256] = qkvT[k * 128:(k + 1) * 128,
                                              E + m * 128:E + (m + 1) * 128]
        shared[f"qkw{l}"] = bfl(qk_t)
        vT = qkvT[:, 2 * E:]                 # [E, E]
        v_t = np.ascontiguousarray(
            vT.reshape(EC, 128, 2, 512).transpose(2, 0, 1, 3))
        shared[f"vw{l}"] = bfl(v_t)
        shared[f"bvrow{l}"] = bfl(
            np.asarray(inputs['sa_in_b'][l], np.float32)[2 * E:][None, :])
        shared[f"ow{l}"] = bfl(tile4(
            np.asarray(inputs['sa_out_w'][l], np.float32).T, EC, EC, 128))
        # l1: loops are m-outer over FC, k over EC -> want [m, k, 128, 128]
        l1T = np.asarray(inputs['sa_l1_w'][l], np.float32).T   # [E, DFF]
        l1_t = np.ascontiguousarray(
            l1T.reshape(EC, 128, FC, 128).transpose(2, 0, 1, 3))
        shared[f"l1w{l}"] = bfl(l1_t)
        # l2: [m, k, 128, 128] with k over FC
        l2T = np.asarray(inputs['sa_l2_w'][l], np.float32).T   # [DFF, E]
        l2_t = np.ascontiguousarray(
            l2T.reshape(FC, 128, EC, 128).transpose(2, 0, 1, 3))
        shared[f"l2w{l}"] = bfl(l2_t)

    in_maps = []
    for cc in range(N_CORES):
        m = dict(shared)
        m["objT"] = tr(obj[cc * NS:(cc + 1) * NS])
        w0 = cc * NW
        m["pobjT"] = tr(packed[w0:w0 + NW].reshape(T, E))
        m["selT"] = bfl(np.ascontiguousarray(
            sel[w0:w0 + NW].reshape(T, 68).T))
        m["maskcol"] = np.ascontiguousarray(
            maskb[w0:w0 + NW].reshape(NW * 2, 128).T)
        in_maps.append(m)

    host = dict(NW=NW, NS=NS, bF=bF, nw=nw, cum=cum,
                pcol_ncols=pcol.shape[1], pcol_idx=pcol_idx,
                in_maps=in_maps)
    return host


def kernel(**inputs):
    import concourse.bass_utils as bass_utils

    host = _prepare(inputs)
    key = "nc"
    if key not in _CACHE:
        _CACHE[key] = _build(host)
    nc = _CACHE[key]

    res = bass_utils.run_bass_kernel_spmd(
        nc, host["in_maps"], core_ids=list(range(N_CORES)))
    outs = [r["outD"] for r in res.results]

    NW = host["NW"]
    nw, cum = host["nw"], host["cum"]
    N = np.asarray(inputs['obj_features']).shape[0]
    obj_idx = np.asarray(inputs['obj_idx']).astype(np.int64)
    owin = np.concatenate(
        [np.asarray(o, np.float32).T.reshape(NW, L_WIN, E) for o in outs],
        axis=0)
    win = np.clip(obj_idx - 1, 0, nw - 1)
    slot = np.clip(np.arange(N) - cum[win], 0, L_WIN - 1)
    return np.ascontiguousarray(owin[win, slot]).astype(np.float32)
